# revision 1
# baseline (speedup 1.0000x reference)
"""Trainium2 Bass kernel for nn_AFF_MambaLayer (bi-directional selective scan).

Sharding: one depth-slice (1024 tokens) per core (8 cores). Each core runs
both scan directions for all 192 channels over its slice. The full-sequence
scan is recovered from the per-slice local scans plus a cross-slice carry
correction; carries are exchanged with one small AllGather.

Partition convention: the 192-channel dimension is split 128 + 64, with the
64-row half stored in [128, *] tiles at base partition 64 (hardware compute
ops require all operands to share a start partition in {0, 32, 64}).
"""
import os
import sys

import numpy as np

sys.path.insert(0, "/opt/trn_rl_repo")

# geometry
C = 96
DIN = 192
N = 16
R = 6
NS = 8           # slices == cores
SL = 1024        # tokens per slice
TP = SL + 3      # with conv left-context
NT = 24          # scan tiles per direction (8 d-channels x 16 states each)
WIN = 256        # correction window (carry decay is dead past this)

_cache = {}


def _build_graph(dbg=False):
    import concourse.bass as bass
    import concourse.bacc as bacc
    import concourse.mybir as mybir
    from concourse import tile

    FP32 = mybir.dt.float32
    F32R = mybir.dt.float32r
    AF = mybir.ActivationFunctionType
    OP = mybir.AluOpType

    nc = bacc.Bacc("TRN2", target_bir_lowering=False, debug=False, num_devices=NS)

    P = {}

    def inp(name, shape):
        P[name] = nc.dram_tensor(name, list(shape), FP32, kind="ExternalInput").ap()

    inp("x_sl", [C, TP])
    inp("w1T", [C, 512])
    inp("padfix", [DIN, 3])
    inp("convw", [DIN, 4])
    inp("bias_u", [DIN, 1])
    inp("bias_z", [DIN, 1])
    for w in ("f", "r"):
        inp(f"xprojT_{w}", [DIN, 70])
        inp(f"dtwT_{w}", [70, 256])
        inp(f"dtb_{w}", [DIN, 1])
        inp(f"Arep_{w}", [128, NT])
        inp(f"Asel_{w}", [DIN, 128])
        for m in ("mP", "mP0", "mF"):
            inp(f"{m}_{w}", [128, NT * 8])
    inp("stotmask", [DIN, NT])
    for v in range(8):
        inp(f"ohs{v}", [128, 128])
    for t in range(16):
        inp(f"red128_{t}", [128, 128])
    inp("oh16s", [112, 128])
    inp("ident", [128, 128])
    inp("mean96", [C, C])
    inp("Dsum", [DIN, 1])
    inp("outpT", [DIN, C])
    inp("fuswT", [C, C])
    inp("fusb", [C, 1])

    out_t = nc.dram_tensor("out", [C, SL], FP32, kind="ExternalOutput").ap()
    dbg_t = {}
    if dbg:
        for name, shape in [
            ("dbg_u", [DIN, SL]), ("dbg_g", [DIN, SL]), ("dbg_dt_f", [DIN, SL]),
            ("dbg_srel_f", [DIN, SL]), ("dbg_brep_f", [128, SL]),
            ("dbg_h0_f", [128, SL]), ("dbg_ysl", [DIN, SL]),
            ("dbg_fp", [128, 96]), ("dbg_hf", [128, NT * 8]),
            ("dbg_hr", [128, NT * 8]), ("dbg_ysq", [DIN, SL]),
            ("dbg_osq", [C, SL]), ("dbg_osl", [C, SL]),
        ]:
            dbg_t[name] = nc.dram_tensor(name, shape, FP32, kind="ExternalOutput").ap()

    RG = [list(range(NS))]

    with tile.TileContext(nc) as tc:
        with (
            tc.tile_pool(name="const", bufs=1) as cst,
            tc.tile_pool(name="pers", bufs=1) as pers,
            tc.tile_pool(name="wk", bufs=3) as wk,
            tc.tile_pool(name="psw", bufs=2, space="PSUM") as psw,
            tc.tile_pool(name="dram", bufs=1, space="DRAM") as drp,
        ):
            # x slice first so LayerNorm can start ASAP
            x_sb = pers.tile([C, TP], FP32, name="x_sb", tag="x_sb")
            nc.sync.dma_start(x_sb[:, :], P["x_sl"])

            # ---------------- constants to SBUF ----------------
            cones = {}

            def load(name, rdt=None):
                shp = list(P[name].shape)

                def one(nm, dram_ap, nrows, base):
                    if rdt is None:
                        tf = cst.tile([128, shp[1]], FP32, name=nm, tag=nm)
                        t = tf[base:base + nrows]
                        nc.sync.dma_start(t[:, :], dram_ap)
                        return t
                    stg = wk.tile([128, shp[1]], FP32, name="cstage", tag="cstage",
                                  bufs=4)
                    nc.sync.dma_start(stg[base:base + nrows, :], dram_ap)
                    tf = cst.tile([128, shp[1]], rdt, name=nm, tag=nm)
                    t = tf[base:base + nrows]
                    nc.scalar.copy(t[:, :], stg[base:base + nrows, :])
                    return t

                if shp[0] <= 128:
                    return one(f"c_{name}", P[name], shp[0], 0)
                t0 = one(f"c_{name}_a", P[name][0:128, :], 128, 0)
                t1 = one(f"c_{name}_b", P[name][128:shp[0], :], shp[0] - 128, 64)
                return (t0, t1)

            def rsl(cn, r0, r1):
                """Absolute-row slice of a possibly split constant."""
                if isinstance(cn, tuple):
                    if r1 <= 128:
                        return cn[0][r0:r1]
                    assert r0 >= 128
                    return cn[1][r0 - 128:r1 - 128]
                return cn[r0:r1]

            BF16 = mybir.dt.bfloat16
            for nm in ("mean96", "w1T"):
                cones[nm] = load(nm, rdt=F32R)
            for nm in ("padfix", "convw", "bias_u", "bias_z"):
                cones[nm] = load(nm)
            for w in ("f", "r"):
                for nm in (f"xprojT_{w}", f"dtwT_{w}"):
                    cones[nm] = load(nm, rdt=F32R)
                for nm in (f"dtb_{w}", f"Arep_{w}", f"Asel_{w}", f"mP_{w}",
                           f"mP0_{w}", f"mF_{w}"):
                    cones[nm] = load(nm)
            cones["oh16s"] = load("oh16s", rdt=F32R)
            for nm in ("stotmask", "ident", "Dsum", "fusb"):
                cones[nm] = load(nm)
            for v in range(8):
                cones[f"ohs{v}"] = load(f"ohs{v}", rdt=F32R)
            for nm in ("outpT", "fuswT", *[f"red128_{t}" for t in range(16)]):
                cones[nm] = load(nm, rdt=BF16)

            def v64(pool, name, cols, tag, bufs=None, dt=FP32):
                """[64, cols] logical tile stored at base partition 64."""
                kw = dict(name=name, tag=tag)
                if bufs is not None:
                    kw["bufs"] = bufs
                t = pool.tile([128, cols], dt, **kw)
                return t[64:128]

            # persistent activations (second halves at base 64)
            g0 = pers.tile([128, SL], FP32, name="g0", tag="g0")
            g1 = v64(pers, "g1", SL, "g1")
            u0 = pers.tile([128, SL], F32R, name="u0", tag="u0")
            u1 = v64(pers, "u1", SL, "u1", dt=F32R)
            u0f = u0.bitcast(FP32)
            u1f = u1.bitcast(FP32)

            # ---------------- preprocessing (scoped pool) ----------------
            with tc.tile_pool(name="pre", bufs=1) as pre:
                xsq = pre.tile([C, TP], F32R, name="xsq", tag="xsq")
                nc.scalar.square(xsq[:, :], x_sb[:, :])
                x_r = pre.tile([C, TP], F32R, name="x_r", tag="x_r")
                nc.scalar.copy(x_r[:, :], x_sb[:, :])
                mu_b = pre.tile([C, TP], FP32, name="mu_b", tag="mu_b")
                msq_b = pre.tile([C, TP], FP32, name="msq_b", tag="msq_b")
                for c0, cn in ((0, 512), (512, 512), (1024, 3)):
                    ps1 = psw.tile([C, cn], FP32, name="ln1_ps", tag="w", space="PSUM")
                    mcast = (lambda a: a.bitcast(FP32)) if cn < 256 else (lambda a: a)
                    nc.tensor.matmul(ps1[:, :], mcast(cones["mean96"][:, :]),
                                     mcast(x_r[:, c0:c0 + cn]),
                                     start=True, stop=True)
                    nc.scalar.copy(mu_b[:, c0:c0 + cn], ps1[:, :])
                    ps2 = psw.tile([C, cn], FP32, name="ln2_ps", tag="w", space="PSUM")
                    nc.tensor.matmul(ps2[:, :], mcast(cones["mean96"][:, :]),
                                     mcast(xsq[:, c0:c0 + cn]),
                                     start=True, stop=True)
                    nc.scalar.copy(msq_b[:, c0:c0 + cn], ps2[:, :])
                istd_b = pre.tile([C, TP], FP32, name="istd_b", tag="istd_b")
                nc.vector.tensor_mul(istd_b[:, :], mu_b[:, :], mu_b[:, :])
                nc.vector.tensor_sub(istd_b[:, :], msq_b[:, :], istd_b[:, :])
                nc.vector.tensor_scalar_add(istd_b[:, :], istd_b[:, :], 1e-5)
                nc.scalar.activation(istd_b[:, :], istd_b[:, :], AF.Sqrt)
                nc.vector.reciprocal(istd_b[:, :], istd_b[:, :])
                xn = pre.tile([C, TP], F32R, name="xn", tag="xn")
                nc.vector.tensor_sub(xn[:, :], x_sb[:, :], mu_b[:, :])
                nc.vector.tensor_mul(xn[:, :], xn[:, :].bitcast(FP32), istd_b[:, :])

                # -------- in_proj GEMM (4 M-tiles, base-aligned pieces) ----
                u_raw0 = pre.tile([128, TP], FP32, name="u_raw0", tag="u_raw0")
                u_raw1 = v64(pre, "u_raw1", TP, "u_raw1")
                w1T = cones["w1T"]
                bz = cones["bias_z"]
                for c0, cn in ((0, 512), (512, 512), (1024, 3)):
                    if c0 == 0:
                        gw0, gw1, pw = 0, 509, 3
                    elif c0 == 512:
                        gw0, gw1, pw = 509, 1021, 0
                    else:
                        gw0, gw1, pw = 1021, 1024, 0
                    for m in range(4):
                        ps = psw.tile([128, cn], FP32, name="xz_ps", tag="w",
                                      space="PSUM")
                        mcast = ((lambda a: a.bitcast(FP32)) if cn < 256
                                 else (lambda a: a))
                        nc.tensor.matmul(ps[:, :],
                                         mcast(w1T[:, m * 128:(m + 1) * 128]),
                                         mcast(xn[:, c0:c0 + cn]),
                                         start=True, stop=True)
                        if m == 0:
                            nc.scalar.copy(u_raw0[:, c0:c0 + cn], ps[:, :])
                        elif m == 1:
                            nc.scalar.activation(g0[0:64, gw0:gw1], ps[0:64, pw:cn],
                                                 AF.Silu, bias=rsl(bz, 0, 64)[:, 0:1])
                            nc.scalar.copy(u_raw1[:, c0:c0 + cn], ps[64:128, :])
                        elif m == 2:
                            nc.scalar.activation(g0[64:128, gw0:gw1],
                                                 ps[64:128, pw:cn], AF.Silu,
                                                 bias=rsl(bz, 64, 128)[:, 0:1])
                        else:
                            nc.scalar.activation(g1[:, gw0:gw1], ps[64:128, pw:cn],
                                                 AF.Silu,
                                                 bias=rsl(bz, 128, 192)[:, 0:1])

                nc.vector.tensor_add(u_raw0[:, 0:3], u_raw0[:, 0:3],
                                     rsl(cones["padfix"], 0, 128)[:, :])
                nc.vector.tensor_add(u_raw1[:, 0:3], u_raw1[:, 0:3],
                                     rsl(cones["padfix"], 128, 192)[:, :])

                # -------- causal conv + SiLU --------
                convw = cones["convw"]
                for (urw, usb, r0, nr) in ((u_raw0, u0, 0, 128), (u_raw1, u1, 128, 64)):
                    cw = lambda k: rsl(convw, r0, r0 + nr)[:, k:k + 1]
                    if nr == 128:
                        tmp = pre.tile([nr, SL], FP32, name=f"cva{r0}", tag=f"cva{r0}")
                        tmp2 = pre.tile([nr, SL], FP32, name=f"cvb{r0}", tag=f"cvb{r0}")
                    else:
                        tmp = v64(pre, f"cva{r0}", SL, f"cva{r0}")
                        tmp2 = v64(pre, f"cvb{r0}", SL, f"cvb{r0}")
                    nc.vector.tensor_scalar_mul(tmp[:, :], urw[0:nr, 0:SL], cw(0))
                    nc.vector.scalar_tensor_tensor(tmp2[:, :], urw[0:nr, 1:SL + 1],
                                                   cw(1), tmp[:, :], OP.mult, OP.add)
                    nc.vector.scalar_tensor_tensor(tmp[:, :], urw[0:nr, 2:SL + 2],
                                                   cw(2), tmp2[:, :], OP.mult, OP.add)
                    nc.vector.scalar_tensor_tensor(tmp2[:, :], urw[0:nr, 3:SL + 3],
                                                   cw(3), tmp[:, :], OP.mult, OP.add)
                    nc.scalar.activation(usb[:, :], tmp2[:, :], AF.Silu,
                                         bias=rsl(cones["bias_u"], r0, r0 + nr)[:, 0:1])

            if dbg:
                nc.sync.dma_start(dbg_t["dbg_u"][0:128, :], u0f[:, :])
                nc.sync.dma_start(dbg_t["dbg_u"][128:192, :], u1f[:, :])
                nc.sync.dma_start(dbg_t["dbg_g"][0:128, :], g0[:, :])
                nc.sync.dma_start(dbg_t["dbg_g"][128:192, :], g1[:, :])

            # zeros helper for cumsum scans
            zc = cst.tile([128, SL], FP32, name="zc", tag="zc")
            nc.vector.memset(zc[:, :], 0.0)

            # per-direction F/Q collection buffers: F cols 0..23, Q 24..47
            fp_d = {w: pers.tile([128, 48], FP32, name=f"fp_{w}", tag=f"fp_{w}")
                    for w in ("f", "r")}

            # correction-phase persistents (windowed)
            srw0_t = {w: pers.tile([128, WIN], F32R, name=f"srw0_{w}", tag=f"srw0_{w}")
                      for w in ("f", "r")}
            srw1_t = {w: v64(pers, f"srw1_{w}", WIN, f"srw1_{w}", dt=F32R)
                      for w in ("f", "r")}
            crw_t = {w: pers.tile([128, WIN], BF16, name=f"crw_{w}", tag=f"crw_{w}")
                     for w in ("f", "r")}

            ag_in = {w: drp.tile([48, 128], FP32, name=f"ag_in_{w}", tag=f"ag_in_{w}",
                                 space="DRAM") for w in ("f", "r")}
            ag_out = {w: drp.tile([NS, 48 * 128], FP32, name=f"ag_out_{w}",
                                  tag=f"ag_out_{w}", space="DRAM",
                                  addr_space="Shared") for w in ("f", "r")}
            h_in = {}
            ycsb = {}

            with (
                tc.tile_pool(name="dirp", bufs=1) as dirp,
                tc.tile_pool(name="psy", bufs=1, space="PSUM") as psy,
                tc.tile_pool(name="psxc", bufs=1, space="PSUM") as psxc,
                tc.tile_pool(name="psxy", bufs=1, space="PSUM") as psxy,
            ):
                y_ps0 = psy.tile([128, SL], FP32, name="y_ps0", tag="y0", space="PSUM")
                y_ps1f = psy.tile([128, SL], FP32, name="y_ps1", tag="y1", space="PSUM")
                y_ps1 = y_ps1f[64:128]
                yslg0 = pers.tile([128, SL], BF16, name="yslg0", tag="yslg0")
                yslg1 = v64(pers, "yslg1", SL, "yslg1", dt=BF16)

                for di, w in enumerate(("f", "r")):
                    rev = (w == "r")
                    # xproj GEMM -> pT [70, 1024] (B:0..15, C:32..47, dt:64..69)
                    pT = dirp.tile([70, SL], F32R, name=f"pT_{w}", tag="pT")
                    xpT = cones[f"xprojT_{w}"]
                    for c0 in (0, 512):
                        ps = psw.tile([70, 512], FP32, name="pt_ps", tag="w",
                                      space="PSUM")
                        nc.tensor.matmul(ps[:, :],
                                         rsl(xpT, 0, 128)[:, :],
                                         u0[:, c0:c0 + 512],
                                         start=True, stop=False)
                        nc.tensor.matmul(ps[:, :],
                                         rsl(xpT, 128, 192)[:, :],
                                         u1[:, c0:c0 + 512],
                                         start=False, stop=True)
                        nc.scalar.copy(pT[:, c0:c0 + 512], ps[:, :])

                    # dt = softplus(dtw @ p_dt + dtb) = ln(1 + exp(.))
                    dt0 = dirp.tile([128, SL], F32R, name=f"dt0_{w}", tag="dt0")
                    dt1 = v64(dirp, f"dt1_{w}", SL, "dt1", dt=F32R)
                    dtwT = cones[f"dtwT_{w}"]
                    dtb = cones[f"dtb_{w}"]
                    for (dst, r0, nr, po, l0) in ((dt0, 0, 128, 0, 0),
                                                  (dt1, 128, 64, 64, 128)):
                        for c0 in (0, 512):
                            ps = psw.tile([128, 512], FP32, name="dt_ps", tag="w",
                                          space="PSUM")
                            nc.tensor.matmul(ps[:, :],
                                             dtwT[64:70, l0:l0 + 128],
                                             pT[64:70, c0:c0 + 512],
                                             start=True, stop=True)
                            spt = wk.tile([128, 512], FP32, name="spt", tag="spt")
                            nc.scalar.activation(spt[po:po + nr, :], ps[po:po + nr, :],
                                                 AF.Exp,
                                                 bias=rsl(dtb, r0, r0 + nr)[:, 0:1])
                            nc.scalar.activation(dst[:, c0:c0 + 512],
                                                 spt[po:po + nr, :], AF.Ln, bias=1.0)

                    # dtu = dt * u
                    dtu0 = dirp.tile([128, SL], F32R, name=f"dtu0_{w}", tag="dtu0")
                    dtu1 = v64(dirp, f"dtu1_{w}", SL, "dtu1", dt=F32R)
                    nc.vector.tensor_mul(dtu0[:, :], dt0[:, :].bitcast(FP32), u0f[:, :])
                    nc.vector.tensor_mul(dtu1[:, :], dt1[:, :].bitcast(FP32), u1f[:, :])

                    # Srel = inclusive cumsum of dt along t (direction-aware)
                    sr0 = dirp.tile([128, SL], FP32, name=f"sr0_{w}", tag="sr0")
                    sr1 = v64(dirp, f"sr1_{w}", SL, "sr1")
                    for (srct, dstt, nr, po) in ((dt0.bitcast(FP32), sr0, 128, 0),
                                                 (dt1.bitcast(FP32), sr1, 64, 64)):
                        zs = zc[po:po + nr, :]
                        if rev:
                            nc.vector.tensor_tensor_scan(
                                dstt[0:nr, ::-1], srct[0:nr, ::-1], zs, 0.0,
                                OP.add, OP.add)
                        else:
                            nc.vector.tensor_tensor_scan(
                                dstt[0:nr, :], srct[0:nr, :], zs, 0.0,
                                OP.add, OP.add)
                    if dbg and w == "f":
                        nc.sync.dma_start(dbg_t["dbg_dt_f"][0:128, :],
                                          dt0[:, :].bitcast(FP32))
                        nc.sync.dma_start(dbg_t["dbg_dt_f"][128:192, :],
                                          dt1[:, :].bitcast(FP32))
                        nc.sync.dma_start(dbg_t["dbg_srel_f"][0:128, :], sr0[:, :])
                        nc.sync.dma_start(dbg_t["dbg_srel_f"][128:192, :], sr1[:, :])

                    # B/C replicated to 128 rows
                    brep = dirp.tile([128, SL], BF16, name=f"brep_{w}", tag="brep")
                    crep = dirp.tile([128, SL], BF16, name=f"crep_{w}", tag="crep")
                    oh16s = cones["oh16s"]
                    for c0 in (0, 512):
                        psb = psw.tile([128, 512], FP32, name="b_ps", tag="w",
                                       space="PSUM")
                        nc.tensor.matmul(psb[:, :], oh16s[0:16, :],
                                         pT[0:16, c0:c0 + 512],
                                         start=True, stop=True)
                        nc.scalar.copy(brep[:, c0:c0 + 512], psb[:, :])
                        psc2 = psw.tile([128, 512], FP32, name="c_ps", tag="w",
                                        space="PSUM")
                        nc.tensor.matmul(psc2[:, :], oh16s[32:48, :],
                                         pT[32:48, c0:c0 + 512],
                                         start=True, stop=True)
                        nc.scalar.copy(crep[:, c0:c0 + 512], psc2[:, :])
                    if dbg and w == "f":
                        bd = wk.tile([128, SL], FP32, name="bd", tag="ydmp", bufs=2)
                        nc.scalar.copy(bd[:, :], brep[:, :])
                        nc.sync.dma_start(dbg_t["dbg_brep_f"][:, :], bd[:, :])

                    # Q_all = A * Stot  (matmul trick) -> qall cols di*NT..
                    stot_col0 = sr0[:, SL - 1:SL] if not rev else sr0[:, 0:1]
                    stot_col1 = sr1[:, SL - 1:SL] if not rev else sr1[:, 0:1]
                    stm0 = wk.tile([128, NT], FP32, name="stm0", tag="stm0", bufs=2)
                    stm1 = v64(wk, "stm1", NT, "stm1", bufs=2)
                    nc.vector.tensor_scalar_mul(stm0[:, :],
                                                rsl(cones["stotmask"], 0, 128)[:, :],
                                                stot_col0)
                    nc.vector.tensor_scalar_mul(stm1[:, :],
                                                rsl(cones["stotmask"], 128, 192)[:, :],
                                                stot_col1)
                    asel = cones[f"Asel_{w}"]
                    qall = psw.tile([128, NT], FP32, name="qall", tag="w", space="PSUM")
                    nc.tensor.matmul(qall[:, :],
                                     rsl(asel, 0, 128)[:, :],
                                     stm0[:, :], start=True, stop=False)
                    nc.tensor.matmul(qall[:, :],
                                     rsl(asel, 128, 192)[:, :],
                                     stm1[:, :], start=False, stop=True)
                    nc.scalar.copy(fp_d[w][:, NT:2 * NT], qall[:, :])

                    # window copies for the correction phase
                    wv0 = 0 if not rev else SL - WIN
                    nc.scalar.copy(srw0_t[w][:, :], sr0[:, wv0:wv0 + WIN])
                    nc.scalar.copy(srw1_t[w][:, :], sr1[:, wv0:wv0 + WIN])
                    nc.scalar.copy(crw_t[w][:, :], crep[:, wv0:wv0 + WIN])

                    # ---- per-tile scan pipeline ----
                    arep = cones[f"Arep_{w}"]
                    for k in range(NT):
                        r0 = 8 * k
                        (srcdt, srcdtu) = (dt0, dtu0) if r0 < 128 else (dt1, dtu1)
                        ro = r0 if r0 < 128 else r0 - 128
                        q0 = (ro // 64) * 64
                        nq = min(64, (128 if r0 < 128 else 64) - q0)
                        oq = q0 if r0 < 128 else 64   # ohs slice base matches rhs
                        v = (ro % 64) // 8
                        ohs = cones[f"ohs{v}"]
                        dA = wk.tile([128, SL], BF16, name="dA", tag="dA")
                        for c0 in (0, 512):
                            rp = psw.tile([128, 512], FP32, name="rep_ps", tag="w",
                                          space="PSUM")
                            nc.tensor.matmul(rp[:, :],
                                             ohs[oq:oq + nq, :],
                                             srcdt[q0:q0 + nq, c0:c0 + 512],
                                             start=True, stop=True)
                            nc.scalar.activation(dA[:, c0:c0 + 512], rp[:, :], AF.Exp,
                                                 scale=arep[:, k:k + 1])
                        dBu = wk.tile([128, SL], BF16, name="dBu", tag="dBu")
                        for c0 in (0, 512):
                            rp = psw.tile([128, 512], FP32, name="rep2_ps", tag="w",
                                          space="PSUM")
                            nc.tensor.matmul(rp[:, :],
                                             ohs[oq:oq + nq, :],
                                             srcdtu[q0:q0 + nq, c0:c0 + 512],
                                             start=True, stop=True)
                            if k % 2 == 0:
                                nc.vector.tensor_mul(dBu[:, c0:c0 + 512], rp[:, :],
                                                     brep[:, c0:c0 + 512])
                            else:
                                # route via ACT + Pool to unload the DVE
                                dtur = wk.tile([128, 512], BF16, name="dtur",
                                               tag="dtur", bufs=3)
                                nc.scalar.copy(dtur[:, :], rp[:, :])
                                nc.gpsimd.tensor_mul(dBu[:, c0:c0 + 512],
                                                     dtur[:, :],
                                                     brep[:, c0:c0 + 512])
                        h = wk.tile([128, SL], BF16, name="h", tag="h")
                        if rev:
                            nc.vector.tensor_tensor_scan(h[:, ::-1], dA[:, ::-1],
                                                         dBu[:, ::-1], 0.0,
                                                         OP.mult, OP.add)
                        else:
                            nc.vector.tensor_tensor_scan(h[:, :], dA[:, :], dBu[:, :],
                                                         0.0, OP.mult, OP.add)
                        fcol = SL - 1 if not rev else 0
                        nc.vector.tensor_copy(fp_d[w][:, k:k + 1],
                                              h[:, fcol:fcol + 1])
                        if dbg and w == "f" and k == 0:
                            hd = wk.tile([128, SL], FP32, name="hd", tag="ydmp", bufs=2)
                            nc.scalar.copy(hd[:, :], h[:, :])
                            nc.sync.dma_start(dbg_t["dbg_h0_f"][:, :], hd[:, :])
                        # hC = h * Crep  (Pool engine)
                        hC = wk.tile([128, SL], BF16, name="hC", tag="hC")
                        nc.gpsimd.tensor_mul(hC[:, :], h[:, :], crep[:, :])
                        # y reduce, full-height output (dst base must be 0)
                        yps = y_ps0 if r0 < 128 else y_ps1f
                        t = k if r0 < 128 else k - 8
                        kfirst = 0 if r0 < 128 else 16
                        klast = 15 if r0 < 128 else 23
                        for c0 in (0, 512):
                            nc.tensor.matmul(yps[:, c0:c0 + 512],
                                             cones[f"red128_{t}"][:, :],
                                             hC[:, c0:c0 + 512],
                                             start=(di == 0 and k == kfirst),
                                             stop=(di == 1 and k == klast))

                    # ---- per-direction: AG + carry + correction compute ----
                    if dbg and w == "f":
                        nc.sync.dma_start(dbg_t["dbg_fp"][:, 0:48], fp_d[w][:, :])
                    fpt_ps = psw.tile([48, 128], FP32, name="fpt_ps", tag="w",
                                      space="PSUM")
                    nc.tensor.transpose(fpt_ps[:, :], fp_d[w][:, :],
                                        cones["ident"][:, :].bitcast(FP32))
                    fpt_sb = wk.tile([48, 128], FP32, name="fpt_sb", tag="fpt", bufs=2)
                    nc.scalar.copy(fpt_sb[:, :], fpt_ps[:, :])
                    nc.sync.dma_start(ag_in[w][:, :], fpt_sb[:, :])
                    nc.gpsimd.collective_compute(
                        "AllGather", mybir.AluOpType.bypass,
                        ins=[ag_in[w][:, :].opt()],
                        outs=[ag_out[w][:, :].opt()],
                        replica_groups=RG,
                    )
                    # carry: transpose gathered F/Q, masked prefix scan
                    carry_ps = psxc.tile([128, 384], FP32, name=f"carry_{w}",
                                         tag="xc", space="PSUM")
                    for bg in range(8):
                        agc = wk.tile([NS, 6 * 128], FP32, name="agc", tag="agc",
                                      bufs=2)
                        nc.sync.dma_start(agc[:, :],
                                          ag_out[w][:, bg * 768:(bg + 1) * 768])
                        for j in range(6):
                            cblk = 6 * bg + j
                            nc.tensor.transpose(carry_ps[:, 8 * cblk:8 * cblk + 8],
                                                agc[:, 128 * j:128 * j + 128],
                                                cones["ident"][0:8, 0:8].bitcast(FP32))
                    pt = wk.tile([128, NT * 8], FP32, name=f"cp_{w}", tag="cpt", bufs=2)
                    nc.scalar.activation(pt[:, :], carry_ps[:, 192:384], AF.Exp)
                    nc.vector.tensor_mul(pt[:, :], pt[:, :], cones[f"mP_{w}"][:, :])
                    nc.vector.tensor_add(pt[:, :], pt[:, :], cones[f"mP0_{w}"][:, :])
                    ft = wk.tile([128, NT * 8], FP32, name=f"cf_{w}", tag="cft", bufs=2)
                    nc.vector.tensor_mul(ft[:, :], carry_ps[:, 0:192],
                                         cones[f"mF_{w}"][:, :])
                    hsc = pers.tile([128, NT * 8], FP32, name=f"hsc_{w}",
                                    tag=f"hsc_{w}")
                    if w == "f":
                        nc.vector.tensor_tensor_scan(hsc[:, :], pt[:, :], ft[:, :],
                                                     0.0, OP.mult, OP.add)
                    else:
                        nc.vector.tensor_tensor_scan(hsc[:, ::-1], pt[:, ::-1],
                                                     ft[:, ::-1], 0.0, OP.mult, OP.add)
                    h_in[w] = hsc
                    if dbg:
                        nc.sync.dma_start(
                            dbg_t["dbg_hf" if w == "f" else "dbg_hr"][:, :], hsc[:, :])


                # ---- correction compute, both directions (fills scheduling
                # gaps; fwd part overlaps the reverse main loop) ----
                for di, w in enumerate(("f", "r")):
                    rev = (w == "r")
                    arep = cones[f"Arep_{w}"]
                    # correction windows, reduced into one psum bank
                    w0 = 0 if not rev else SL - WIN
                    hofs = 7 if not rev else 0
                    crw = crw_t[w]
                    yc = psxy.tile([128, 512], FP32, name=f"yc_{w}", tag="xy",
                                  space="PSUM")
                    for k in range(NT):
                        r0 = 8 * k
                        src_sr = srw0_t[w] if r0 < 128 else srw1_t[w]
                        ro = r0 if r0 < 128 else r0 - 128
                        q0 = (ro // 64) * 64
                        nq = min(64, (128 if r0 < 128 else 64) - q0)
                        oq = q0 if r0 < 128 else 64
                        v = (ro % 64) // 8
                        srp = psw.tile([128, WIN], FP32, name="srp", tag="w",
                                       space="PSUM")
                        nc.tensor.matmul(srp[:, :], cones[f"ohs{v}"][oq:oq + nq, :],
                                         src_sr[q0:q0 + nq, 0:WIN],
                                         start=True, stop=True)
                        cpda = wk.tile([128, WIN], BF16, name="cpda", tag="cpda",
                                       bufs=12)
                        nc.scalar.activation(cpda[:, :], srp[:, :], AF.Exp,
                                             scale=arep[:, k:k + 1])
                        corr = wk.tile([128, WIN], BF16, name="corr", tag="corr")
                        nc.vector.scalar_tensor_tensor(
                            corr[:, :], cpda[:, :],
                            h_in[w][:, 8 * k + hofs:8 * k + hofs + 1],
                            crw[:, :], OP.mult, OP.mult)
                        yccol = 0 if r0 < 128 else WIN
                        t = k if r0 < 128 else k - 8
                        kfirst = 0 if r0 < 128 else 16
                        klast = 15 if r0 < 128 else 23
                        nc.tensor.matmul(yc[:, yccol:yccol + WIN],
                                         cones[f"red128_{t}"][:, :],
                                         corr[:, :],
                                         start=(k == kfirst), stop=(k == klast))
                    yb0 = pers.tile([128, WIN], BF16, name=f"ycsb0_{w}",
                                    tag=f"ycsb0_{w}")
                    yb1 = v64(pers, f"ycsb1_{w}", WIN, f"ycsb1_{w}", dt=BF16)
                    nc.scalar.copy(yb0[:, :], yc[:, 0:WIN])
                    nc.scalar.copy(yb1[:, :], yc[64:128, WIN:2 * WIN])
                    ycsb[w] = (yb0, yb1)

                # y_sl = y_psum + u*Dsum
                nc.vector.scalar_tensor_tensor(yslg0[:, :], u0f[:, :],
                                               rsl(cones["Dsum"], 0, 128)[:, 0:1],
                                               y_ps0[:, :], OP.mult, OP.add)
                nc.vector.scalar_tensor_tensor(yslg1[:, :], u1f[:, :],
                                               rsl(cones["Dsum"], 128, 192)[:, 0:1],
                                               y_ps1[:, :], OP.mult, OP.add)
                if dbg:
                    ydmp = wk.tile([128, SL], FP32, name="ydmp", tag="ydmp", bufs=2)
                    nc.scalar.copy(ydmp[:, :], yslg0[:, :])
                    nc.sync.dma_start(dbg_t["dbg_ysl"][0:128, :], ydmp[:, :])
                    ydmp2 = wk.tile([128, SL], FP32, name="ydmp2", tag="ydmp", bufs=2)
                    nc.scalar.copy(ydmp2[64:128, :], yslg1[:, :])
                    nc.sync.dma_start(dbg_t["dbg_ysl"][128:192, :], ydmp2[64:128, :])

            # psy/psq/dirp closed; fold in the gate g
            nc.gpsimd.tensor_mul(yslg0[:, :], yslg0[:, :], g0[:, :])
            nc.gpsimd.tensor_mul(yslg1[:, :], yslg1[:, :], g1[:, :])

            # ---- assemble y_sq*g = y_sl*g + correction windows ----
            ysqg0 = pers.tile([128, SL], BF16, name="ysqg0", tag="ysqg0")
            ysqg1 = v64(pers, "ysqg1", SL, "ysqg1", dt=BF16)
            nc.scalar.copy(ysqg0[:, :], yslg0[:, :])
            nc.scalar.copy(ysqg1[:, :], yslg1[:, :])
            for w in ("f", "r"):
                w0 = 0 if w == "f" else SL - WIN
                yb0, yb1 = ycsb[w]
                for (tgt, gsrc, ycb, nr) in ((ysqg0, g0, yb0, 128),
                                             (ysqg1, g1, yb1, 64)):
                    if nr == 128:
                        cg = wk.tile([128, WIN], BF16, name="cg128", tag="cg128",
                                     bufs=2)
                    else:
                        cg = v64(wk, "cg64", WIN, "cg64", bufs=2, dt=BF16)
                    nc.vector.tensor_mul(cg[:, :], ycb[:, :],
                                         gsrc[0:nr, w0:w0 + WIN])
                    nc.vector.tensor_add(tgt[0:nr, w0:w0 + WIN],
                                         tgt[0:nr, w0:w0 + WIN], cg[:, :])

            if dbg:
                yd3 = wk.tile([128, SL], FP32, name="yd3", tag="ydmp", bufs=2)
                nc.scalar.copy(yd3[:, :], ysqg0[:, :])
                nc.sync.dma_start(dbg_t["dbg_ysq"][0:128, :], yd3[:, :])
                yd4 = wk.tile([128, SL], FP32, name="yd4", tag="ydmp", bufs=2)
                nc.scalar.copy(yd4[64:128, :], ysqg1[:, :])
                nc.sync.dma_start(dbg_t["dbg_ysq"][128:192, :], yd4[64:128, :])

            # ---- out_proj GEMMs + fusion (scoped pool) ----
            with tc.tile_pool(name="fin", bufs=1) as fnp:
                outpT = cones["outpT"]
                osq = fnp.tile([C, SL], BF16, name="osq", tag="osq")
                osl = fnp.tile([C, SL], BF16, name="osl", tag="osl")
                for c0 in (0, 512):
                    for (dst, s0, s1) in ((osq, ysqg0, ysqg1), (osl, yslg0, yslg1)):
                        ps = psw.tile([C, 512], FP32, name="op_ps", tag="w",
                                      space="PSUM")
                        nc.tensor.matmul(ps[:, :],
                                         rsl(outpT, 0, 128)[:, :],
                                         s0[:, c0:c0 + 512],
                                         start=True, stop=False)
                        nc.tensor.matmul(ps[:, :],
                                         rsl(outpT, 128, 192)[:, :],
                                         s1[:, c0:c0 + 512],
                                         start=False, stop=True)
                        nc.scalar.copy(dst[:, c0:c0 + 512], ps[:, :])
                if dbg:
                    od1 = wk.tile([C, SL], FP32, name="od1", tag="ydmp", bufs=2)
                    nc.scalar.copy(od1[:, :], osq[:, :])
                    nc.sync.dma_start(dbg_t["dbg_osq"][:, :], od1[:, :])
                    od2 = wk.tile([C, SL], FP32, name="od2", tag="ydmp", bufs=2)
                    nc.scalar.copy(od2[:, :], osl[:, :])
                    nc.sync.dma_start(dbg_t["dbg_osl"][:, :], od2[:, :])

                ssum = fnp.tile([C, SL], BF16, name="ssum", tag="ssum")
                nc.vector.tensor_add(ssum[:, :], osq[:, :], osl[:, :])
                wgt = fnp.tile([C, SL], BF16, name="wgt", tag="wgt")
                for c0 in (0, 512):
                    ps = psw.tile([C, 512], FP32, name="fus_ps", tag="w", space="PSUM")
                    nc.tensor.matmul(ps[:, :], cones["fuswT"][:, :],
                                     ssum[:, c0:c0 + 512],
                                     start=True, stop=True)
                    nc.scalar.activation(wgt[:, c0:c0 + 512], ps[:, :], AF.Sigmoid,
                                         bias=cones["fusb"][:, 0:1])
                # out = w*(osq-osl) + osl + x_skip
                diff = fnp.tile([C, SL], BF16, name="diff", tag="diff")
                nc.vector.tensor_sub(diff[:, :], osq[:, :], osl[:, :])
                nc.vector.tensor_mul(diff[:, :], diff[:, :], wgt[:, :])
                nc.vector.tensor_add(diff[:, :], diff[:, :], osl[:, :])
                fin = fnp.tile([C, SL], FP32, name="fin", tag="fin")
                nc.vector.tensor_add(fin[:, :], diff[:, :], x_sb[:, 3:3 + SL])
                nc.sync.dma_start(out_t, fin[:, :])

    nc.compile()
    return nc, dbg_t


def _host_prep(inputs):
    """Build per-core input maps (weight folds, masks, slices)."""
    f32 = np.float32
    ln_g = np.asarray(inputs["ln_g"], np.float64)
    ln_b = np.asarray(inputs["ln_b"], np.float64)
    W1 = np.asarray(inputs["in_proj_w"], np.float64)
    W1p = (W1 * ln_g[None, :])
    bW = W1 @ ln_b
    conv_w = np.asarray(inputs["conv_w"], np.float64)
    bias_u = np.asarray(inputs["conv_bias"], np.float64) + bW[:DIN] * conv_w.sum(axis=1)
    bias_z = bW[DIN:]

    x = np.asarray(inputs["x"], np.float32).reshape(C, NS * SL)

    # W1big col layout: [u0..127 | z0..63, u128..191 | pad64, z64..127
    #                    | pad64, z128..191]
    W1big = np.zeros((512, C), np.float64)
    W1big[0:128] = W1p[0:128]
    W1big[128:192] = W1p[DIN:DIN + 64]
    W1big[192:256] = W1p[128:192]
    W1big[320:384] = W1p[DIN + 64:DIN + 128]
    W1big[448:512] = W1p[DIN + 128:DIN + 192]

    shared = {
        "w1T": np.ascontiguousarray(W1big.T).astype(f32),
        "convw": conv_w.astype(f32),
        "bias_u": bias_u.astype(f32)[:, None],
        "bias_z": bias_z.astype(f32)[:, None],
        "stotmask": np.asarray(
            [[1.0 if d // 8 == k else 0.0 for k in range(NT)] for d in range(DIN)], f32),
        "oh16s": np.asarray(
            [[1.0 if (q % 32) < 16 and p % 16 == q % 32 else 0.0
              for p in range(128)] for q in range(112)], f32),
        "ident": np.eye(128, dtype=f32),
        "mean96": np.full((C, C), 1.0 / C, f32),
        "Dsum": (np.asarray(inputs["D_f"], np.float64)
                 + np.asarray(inputs["D_r"], np.float64)).astype(f32)[:, None],
        "outpT": np.ascontiguousarray(np.asarray(inputs["out_proj_w"]).T).astype(f32),
        "fuswT": np.ascontiguousarray(np.asarray(inputs["fus_w"]).T).astype(f32),
        "fusb": np.asarray(inputs["fus_b"], f32)[:, None],
    }
    for v in range(8):
        shared[f"ohs{v}"] = np.asarray(
            [[1.0 if (q % 64) == 8 * v + p // 16 else 0.0
              for p in range(128)] for q in range(128)], f32)
    for t in range(16):
        shared[f"red128_{t}"] = np.asarray(
            [[1.0 if j == 8 * t + p // 16 else 0.0
              for j in range(128)] for p in range(128)], f32)
    for w in ("f", "r"):
        xp = np.asarray(inputs[f"xproj_{w}"], np.float64)   # [38, 192]
        xp70 = np.zeros((70, DIN), np.float64)
        xp70[0:16] = xp[R:R + N]           # B
        xp70[32:48] = xp[R + N:R + 2 * N]  # C
        xp70[64:70] = xp[0:R]              # dt projection
        shared[f"xprojT_{w}"] = np.ascontiguousarray(xp70.T).astype(f32)
        dtw70 = np.zeros((70, 256), np.float64)
        dtwt = np.asarray(inputs[f"dt_w_{w}"], np.float64).T   # [6, 192]
        dtw70[64:70, 0:128] = dtwt[:, 0:128]
        dtw70[64:70, 192:256] = dtwt[:, 128:192]
        shared[f"dtwT_{w}"] = dtw70.astype(f32)
        shared[f"dtb_{w}"] = np.asarray(inputs[f"dt_b_{w}"], f32)[:, None]
        A = -np.exp(np.asarray(inputs[f"A_log_{w}"], np.float64))  # [DIN, N]
        arep = np.zeros((128, NT), f32)
        asel = np.zeros((DIN, 128), f32)
        for p in range(128):
            for k in range(NT):
                arep[p, k] = A[8 * k + p // 16, p % 16]
            for d in range(DIN):
                if d % 8 == p // 16:
                    asel[d, p] = A[d, p % 16]
        shared[f"Arep_{w}"] = arep
        shared[f"Asel_{w}"] = asel

    in_maps = []
    for s in range(NS):
        m = dict(shared)
        xs = np.zeros((C, TP), f32)
        lo = s * SL - 3
        if lo < 0:
            xs[:, 3:] = x[:, 0:SL]
        else:
            xs[:, :] = x[:, lo:(s + 1) * SL]
        m["x_sl"] = xs
        pf = np.zeros((DIN, 3), f32)
        if s == 0:
            pf[:, :] = np.float32(-bW[:DIN, None])
        m["padfix"] = pf
        for w in ("f", "r"):
            mP = np.zeros(8, f32)
            mP0 = np.zeros(8, f32)
            mF = np.zeros(8, f32)
            for j in range(8):
                if w == "f":
                    mP[j] = 1.0 if 1 <= j < s else 0.0
                    mP0[j] = 1.0 if j >= s else 0.0
                    mF[j] = 1.0 if j < s else 0.0
                else:
                    mP[j] = 1.0 if (s < j <= 6) else 0.0
                    mP0[j] = 1.0 if j <= s else 0.0
                    mF[j] = 1.0 if j > s else 0.0
            m[f"mP_{w}"] = np.tile(np.tile(mP, NT)[None, :], (128, 1)).astype(f32)
            m[f"mP0_{w}"] = np.tile(np.tile(mP0, NT)[None, :], (128, 1)).astype(f32)
            m[f"mF_{w}"] = np.tile(np.tile(mF, NT)[None, :], (128, 1)).astype(f32)
        in_maps.append(m)
    return in_maps


def run_cores(inputs, dbg=False, trace=False):
    from concourse.bass_utils import run_bass_kernel_spmd
    key = ("g", dbg)
    if key not in _cache:
        _cache[key] = _build_graph(dbg=dbg)
    nc, dbg_t = _cache[key]
    in_maps = _host_prep(inputs)
    res = run_bass_kernel_spmd(nc, in_maps, core_ids=list(range(NS)), trace=trace)
    return res, dbg_t


def kernel(**inputs):
    res, _ = run_cores(inputs, dbg=False, trace=False)
    out = np.zeros((C, NS * SL), np.float32)
    for s in range(NS):
        out[:, s * SL:(s + 1) * SL] = res.results[s]["out"]
    return out.reshape(1, C, 8, 32, 32)



# revision 23
# speedup vs baseline: 1.0816x; 1.0816x over previous
"""Trainium2 Bass kernel for nn_AFF_MambaLayer (bi-directional selective scan).

Sharding: one depth-slice (1024 tokens) per core (8 cores). Each core runs
both scan directions for all 192 channels over its slice. Cross-slice decay
is numerically dead (exp(-~0.55*1024)), so the full-sequence scan equals the
local scan plus a WIN-token boundary correction driven by the *neighbor*
core's final state only; finals are exchanged with two small AllGathers.

Partition convention: the 192-channel dimension is split 128 + 64, with the
64-row half stored in [128, *] tiles at base partition 64 (hardware compute
ops require all operands to share a start partition in {0, 32, 64}).
"""
import os
import sys

import numpy as np

sys.path.insert(0, "/opt/trn_rl_repo")

# geometry
C = 96
DIN = 192
N = 16
R = 6
NS = 8           # slices == cores
SL = 1024        # tokens per slice
TP = SL + 3      # with conv left-context
NT = 24          # scan tiles per direction (8 d-channels x 16 states each)
WIN = 128        # correction window (carry decay is dead past ~64)

_cache = {}

# ---- const blob layouts (col offsets), shared host/device ----
_F32_ITEMS = [
    ("padfix_a", 3), ("padfix_b", 3), ("convw_a", 4), ("convw_b", 4),
    ("bias_u_a", 1), ("bias_u_b", 1), ("bias_z_a", 1), ("bias_z_b", 1),
    ("dtb_f_a", 1), ("dtb_f_b", 1), ("dtb_r_a", 1), ("dtb_r_b", 1),
    ("Arep_f", NT), ("Arep_r", NT),
    ("sel_f", 1), ("sel_r", 1),
    ("ident", 128), ("Dsum_a", 1), ("Dsum_b", 1), ("fusb", 1),
]
_F32_OFF = {}
_off = 0
for _nm, _nc in _F32_ITEMS:
    _F32_OFF[_nm] = (_off, _nc)
    _off += _nc
F32_COLS = _off

# f32r blob: constants consumed as matmul stationaries
_F32R_ITEMS = [
    ("oh16s", 128),
    ("xprojT_f_a", 70), ("xprojT_f_b", 70),
    ("xprojT_r_a", 70), ("xprojT_r_b", 70),
    ("dtwT_f", 256), ("dtwT_r", 256),
] + [(f"ohs{v}", 128) for v in range(8)]
_F32R_OFF = {}
_off = 0
for _nm, _nc in _F32R_ITEMS:
    _F32R_OFF[_nm] = (_off, _nc)
    _off += _nc
F32R_COLS = _off

_BF_ITEMS = [("outpT_a", C), ("outpT_b", C), ("fuswT", C)] + [
    (f"red128b_{t}", 128) for t in range(16)]
_BF_OFF = {}
_off = 0
for _nm, _nc in _BF_ITEMS:
    _BF_OFF[_nm] = (_off, _nc)
    _off += _nc
BF_COLS = _off


def _build_graph(dbg=False):
    import concourse.bass as bass
    import concourse.bacc as bacc
    import concourse.mybir as mybir
    from concourse import tile

    FP32 = mybir.dt.float32
    F32R = mybir.dt.float32r
    BF16 = mybir.dt.bfloat16
    AF = mybir.ActivationFunctionType
    OP = mybir.AluOpType

    nc = bacc.Bacc("TRN2", target_bir_lowering=False, debug=False, num_devices=NS)

    P = {}

    def inp(name, shape, dt=FP32):
        P[name] = nc.dram_tensor(name, list(shape), dt, kind="ExternalInput").ap()

    inp("x_sl", [C, TP], F32R)
    inp("mean96", [C, C], F32R)
    inp("w1T", [C, 512], F32R)
    inp("blobf", [128, F32_COLS])
    inp("blobr", [128, F32R_COLS], F32R)
    inp("blobb", [128, BF_COLS], BF16)

    out_t = nc.dram_tensor("out", [C, SL], FP32, kind="ExternalOutput").ap()
    dbg_t = {}
    if dbg:
        for name, shape in [
            ("dbg_u", [DIN, SL]), ("dbg_g", [DIN, SL]), ("dbg_dt_f", [DIN, SL]),
            ("dbg_h0_f", [128, SL]), ("dbg_yslg", [DIN, SL]),
            ("dbg_hin_f", [128, NT]), ("dbg_hin_r", [128, NT]),
            ("dbg_yc_f", [128, 2 * WIN]), ("dbg_osl", [C, SL]),
        ]:
            dbg_t[name] = nc.dram_tensor(name, shape, FP32, kind="ExternalOutput").ap()

    RG = [list(range(NS))]

    with tile.TileContext(nc) as tc:
        with (
            tc.tile_pool(name="const", bufs=1) as cst,
            tc.tile_pool(name="pers", bufs=1) as pers,
            tc.tile_pool(name="wk", bufs=3) as wk,
            tc.tile_pool(name="psw", bufs=2, space="PSUM") as psw,
            tc.tile_pool(name="psc", bufs=1, space="PSUM") as psc,
            tc.tile_pool(name="dram", bufs=1, space="DRAM") as drp,
        ):
            # x slice first so LayerNorm can start ASAP
            x_sbr = pers.tile([C, TP], F32R, name="x_sb", tag="x_sb")
            nc.sync.dma_start(x_sbr[:, :], P["x_sl"])
            x_sb = x_sbr.bitcast(FP32)
            mean96 = cst.tile([C, C], F32R, name="mean96", tag="mean96")
            nc.sync.dma_start(mean96[:, :], P["mean96"])
            w1T = cst.tile([C, 512], F32R, name="w1T", tag="w1T")
            nc.sync.dma_start(w1T[:, :], P["w1T"])
            blobf = cst.tile([128, F32_COLS], FP32, name="blobf", tag="blobf")
            nc.sync.dma_start(blobf[:, :], P["blobf"])
            blobr = cst.tile([128, F32R_COLS], F32R, name="blobr", tag="blobr")
            nc.sync.dma_start(blobr[:, :], P["blobr"])
            blobb = cst.tile([128, BF_COLS], BF16, name="blobb", tag="blobb")
            nc.sync.dma_start(blobb[:, :], P["blobb"])

            def cf(nm, rows=None):
                o, ncol = _F32_OFF[nm]
                t = blobf[:, o:o + ncol] if rows is None else \
                    blobf[rows[0]:rows[1], o:o + ncol]
                return t

            def cfr(nm, rows=None):
                o, ncol = _F32R_OFF[nm]
                return blobr[:, o:o + ncol] if rows is None else \
                    blobr[rows[0]:rows[1], o:o + ncol]

            def cb(nm, rows=None):
                o, ncol = _BF_OFF[nm]
                return blobb[:, o:o + ncol] if rows is None else \
                    blobb[rows[0]:rows[1], o:o + ncol]

            def v64(pool, name, cols, tag, bufs=None, dt=FP32):
                """[64, cols] logical tile stored at base partition 64."""
                kw = dict(name=name, tag=tag)
                if bufs is not None:
                    kw["bufs"] = bufs
                t = pool.tile([128, cols], dt, **kw)
                return t[64:128]

            # persistent activations (second halves at base 64)
            g0 = pers.tile([128, SL], BF16, name="g0", tag="g0")
            g1 = v64(pers, "g1", SL, "g1", dt=BF16)
            u0 = pers.tile([128, SL], F32R, name="u0", tag="u0")
            u1 = v64(pers, "u1", SL, "u1", dt=F32R)
            u0f = u0.bitcast(FP32)
            u1f = u1.bitcast(FP32)

            # ---------------- preprocessing (scoped pool) ----------------
            with tc.tile_pool(name="pre", bufs=1) as pre:
                xsq = pre.tile([C, TP], F32R, name="xsq", tag="xsq")
                nc.scalar.square(xsq[:, :], x_sb[:, :])
                x_r = x_sbr
                mu_b = pre.tile([C, TP], FP32, name="mu_b", tag="mu_b")
                msq_b = pre.tile([C, TP], FP32, name="msq_b", tag="msq_b")
                for c0, cn in ((0, 512), (512, 512), (1024, 3)):
                    ps1 = psw.tile([C, cn], FP32, name="ln1_ps", tag="w", space="PSUM")
                    mcast = (lambda a: a.bitcast(FP32)) if cn < 256 else (lambda a: a)
                    nc.tensor.matmul(ps1[:, :], mcast(mean96[:, :]),
                                     mcast(x_r[:, c0:c0 + cn]),
                                     start=True, stop=True)
                    nc.scalar.copy(mu_b[:, c0:c0 + cn], ps1[:, :])
                    ps2 = psw.tile([C, cn], FP32, name="ln2_ps", tag="w", space="PSUM")
                    nc.tensor.matmul(ps2[:, :], mcast(mean96[:, :]),
                                     mcast(xsq[:, c0:c0 + cn]),
                                     start=True, stop=True)
                    nc.scalar.copy(msq_b[:, c0:c0 + cn], ps2[:, :])
                istd_b = pre.tile([C, TP], FP32, name="istd_b", tag="istd_b")
                nc.vector.tensor_mul(istd_b[:, :], mu_b[:, :], mu_b[:, :])
                nc.vector.tensor_sub(istd_b[:, :], msq_b[:, :], istd_b[:, :])
                nc.vector.tensor_scalar_add(istd_b[:, :], istd_b[:, :], 1e-5)
                nc.scalar.activation(istd_b[:, :], istd_b[:, :], AF.Sqrt)
                nc.vector.reciprocal(istd_b[:, :], istd_b[:, :])
                xn = pre.tile([C, TP], F32R, name="xn", tag="xn")
                nc.vector.tensor_sub(xn[:, :], x_sb[:, :], mu_b[:, :])
                nc.vector.tensor_mul(xn[:, :], xn[:, :].bitcast(FP32), istd_b[:, :])

                # -------- in_proj GEMM (4 M-tiles, base-aligned pieces) ----
                u_raw0 = pre.tile([128, TP], FP32, name="u_raw0", tag="u_raw0")
                u_raw1 = v64(pre, "u_raw1", TP, "u_raw1")
                for c0, cn in ((0, 512), (512, 512), (1024, 3)):
                    if c0 == 0:
                        gw0, gw1, pw = 0, 509, 3
                    elif c0 == 512:
                        gw0, gw1, pw = 509, 1021, 0
                    else:
                        gw0, gw1, pw = 1021, 1024, 0
                    for m in range(4):
                        ps = psw.tile([128, cn], FP32, name="xz_ps", tag="w",
                                      space="PSUM")
                        mcast = ((lambda a: a.bitcast(FP32)) if cn < 256
                                 else (lambda a: a))
                        nc.tensor.matmul(ps[:, :],
                                         mcast(w1T[:, m * 128:(m + 1) * 128]),
                                         mcast(xn[:, c0:c0 + cn]),
                                         start=True, stop=True)
                        if m == 0:
                            nc.scalar.copy(u_raw0[:, c0:c0 + cn], ps[:, :])
                        elif m == 1:
                            nc.scalar.activation(g0[0:64, gw0:gw1], ps[0:64, pw:cn],
                                                 AF.Silu,
                                                 bias=cf("bias_z_a", (0, 64))[:, 0:1])
                            nc.scalar.copy(u_raw1[:, c0:c0 + cn], ps[64:128, :])
                        elif m == 2:
                            nc.scalar.activation(g0[64:128, gw0:gw1],
                                                 ps[64:128, pw:cn], AF.Silu,
                                                 bias=cf("bias_z_a", (64, 128))[:, 0:1])
                        else:
                            nc.scalar.activation(g1[:, gw0:gw1], ps[64:128, pw:cn],
                                                 AF.Silu,
                                                 bias=cf("bias_z_b", (64, 128))[:, 0:1])

                nc.vector.tensor_add(u_raw0[:, 0:3], u_raw0[:, 0:3],
                                     cf("padfix_a")[:, :])
                nc.vector.tensor_add(u_raw1[:, 0:3], u_raw1[:, 0:3],
                                     cf("padfix_b", (64, 128))[:, :])

                # -------- causal conv + SiLU --------
                for (urw, usb, r0, nr, sfx) in ((u_raw0, u0, 0, 128, "a"),
                                                (u_raw1, u1, 128, 64, "b")):
                    cwn = f"convw_{sfx}"
                    bun = f"bias_u_{sfx}"
                    rows = None if nr == 128 else (64, 128)
                    cw = lambda k: cf(cwn, rows)[:, k:k + 1]
                    if nr == 128:
                        tmp = pre.tile([nr, SL], FP32, name=f"cva{r0}", tag="cva")
                        tmp2 = pre.tile([nr, SL], FP32, name=f"cvb{r0}", tag="cvb")
                    else:
                        tmp = v64(pre, f"cva{r0}", SL, "cva")
                        tmp2 = v64(pre, f"cvb{r0}", SL, "cvb")
                    nc.vector.tensor_scalar_mul(tmp[:, :], urw[0:nr, 0:SL], cw(0))
                    nc.vector.scalar_tensor_tensor(tmp2[:, :], urw[0:nr, 1:SL + 1],
                                                   cw(1), tmp[:, :], OP.mult, OP.add)
                    nc.vector.scalar_tensor_tensor(tmp[:, :], urw[0:nr, 2:SL + 2],
                                                   cw(2), tmp2[:, :], OP.mult, OP.add)
                    nc.vector.scalar_tensor_tensor(tmp2[:, :], urw[0:nr, 3:SL + 3],
                                                   cw(3), tmp[:, :], OP.mult, OP.add)
                    nc.scalar.activation(usb[:, :], tmp2[:, :], AF.Silu,
                                         bias=cf(bun, rows)[:, 0:1])

            if dbg:
                nc.sync.dma_start(dbg_t["dbg_u"][0:128, :], u0f[:, :])
                nc.sync.dma_start(dbg_t["dbg_u"][128:192, :], u1f[:, :])
                gd = wk.tile([128, SL], FP32, name="gd", tag="ydmp", bufs=2)
                nc.scalar.copy(gd[:, :], g0[:, :])
                nc.sync.dma_start(dbg_t["dbg_g"][0:128, :], gd[:, :])
                gd2 = wk.tile([128, SL], FP32, name="gd2", tag="ydmp", bufs=2)
                nc.scalar.copy(gd2[64:128, :], g1[:, :])
                nc.sync.dma_start(dbg_t["dbg_g"][128:192, :], gd2[64:128, :])

            # zeros for the window cumsum scans
            zcw = cst.tile([128, WIN], FP32, name="zcw", tag="zcw")
            nc.vector.memset(zcw[:, :], 0.0)

            # ---------------- per-direction preambles (both hoisted) -------
            dirs = ("f", "r")
            dt0_d, dt1_d, dtu0_d, dtu1_d = {}, {}, {}, {}
            brep_d, crep_d, srw0_d, srw1_d, crw_d, fp_d = {}, {}, {}, {}, {}, {}
            for w in dirs:
                rev = (w == "r")
                # xproj GEMM -> pT [70, 1024] (B:0..15, C:32..47, dt:64..69)
                pT = pers.tile([70, SL], F32R, name=f"pT_{w}", tag="pT")
                for c0 in (0, 512):
                    ps = psw.tile([70, 512], FP32, name="pt_ps", tag="w",
                                  space="PSUM")
                    nc.tensor.matmul(ps[:, :],
                                     cfr(f"xprojT_{w}_a")[:, :],
                                     u0[:, c0:c0 + 512],
                                     start=True, stop=False)
                    nc.tensor.matmul(ps[:, :],
                                     cfr(f"xprojT_{w}_b", (64, 128))[:, :],
                                     u1[:, c0:c0 + 512],
                                     start=False, stop=True)
                    nc.scalar.copy(pT[:, c0:c0 + 512], ps[:, :])

                # dt = softplus(dtw @ p_dt + dtb) = ln(1 + exp(.))
                dt0 = pers.tile([128, SL], F32R, name=f"dt0_{w}", tag=f"dt0_{w}")
                dt1 = v64(pers, f"dt1_{w}", SL, f"dt1_{w}", dt=F32R)
                for (dst, po, l0, sfx) in ((dt0, 0, 0, "a"), (dt1, 64, 128, "b")):
                    rows = None if po == 0 else (64, 128)
                    for c0 in (0, 512):
                        ps = psw.tile([128, 512], FP32, name="dt_ps", tag="w",
                                      space="PSUM")
                        nc.tensor.matmul(ps[:, :],
                                         cfr(f"dtwT_{w}", (64, 70))[:, l0:l0 + 128],
                                         pT[64:70, c0:c0 + 512],
                                         start=True, stop=True)
                        spt = wk.tile([128, 512], FP32, name="spt", tag="spt",
                                      bufs=2)
                        nc.scalar.activation(spt[po:128, :], ps[po:128, :], AF.Exp,
                                             bias=cf(f"dtb_{w}_{sfx}", rows)[:, 0:1])
                        nc.scalar.activation(dst[:, c0:c0 + 512],
                                             spt[po:128, :], AF.Ln, bias=1.0)

                # dtu = dt * u
                dtu0 = pers.tile([128, SL], F32R, name=f"dtu0_{w}", tag=f"dtu0_{w}")
                dtu1 = v64(pers, f"dtu1_{w}", SL, f"dtu1_{w}", dt=F32R)
                nc.vector.tensor_mul(dtu0[:, :], dt0[:, :].bitcast(FP32), u0f[:, :])
                nc.vector.tensor_mul(dtu1[:, :], dt1[:, :].bitcast(FP32), u1f[:, :])

                # windowed Srel = inclusive cumsum of dt (direction-aware)
                w0 = 0 if not rev else SL - WIN
                srw0 = pers.tile([128, WIN], F32R, name=f"srw0_{w}", tag=f"srw0_{w}")
                srw1 = v64(pers, f"srw1_{w}", WIN, f"srw1_{w}", dt=F32R)
                for (srct, dstt, nr, po) in ((dt0.bitcast(FP32), srw0, 128, 0),
                                             (dt1.bitcast(FP32), srw1, 64, 64)):
                    zs = zcw[po:po + nr, :]
                    win = srct[0:nr, w0:w0 + WIN]
                    dwin = dstt[0:nr, :]
                    if rev:
                        nc.vector.tensor_tensor_scan(
                            dwin[:, ::-1], win[:, ::-1], zs, 0.0,
                            OP.add, OP.add)
                    else:
                        nc.vector.tensor_tensor_scan(
                            dwin[:, :], win[:, :], zs, 0.0, OP.add, OP.add)

                # B/C replicated to 128 rows
                brep = pers.tile([128, SL], BF16, name=f"brep_{w}", tag=f"brep_{w}")
                crep = pers.tile([128, SL], BF16, name=f"crep_{w}", tag=f"crep_{w}")
                for c0 in (0, 512):
                    psb = psw.tile([128, 512], FP32, name="b_ps", tag="w",
                                   space="PSUM")
                    nc.tensor.matmul(psb[:, :], cfr("oh16s")[0:16, :],
                                     pT[0:16, c0:c0 + 512],
                                     start=True, stop=True)
                    nc.scalar.copy(brep[:, c0:c0 + 512], psb[:, :])
                    psc2 = psw.tile([128, 512], FP32, name="c_ps", tag="w",
                                    space="PSUM")
                    nc.tensor.matmul(psc2[:, :], cfr("oh16s")[32:48, :],
                                     pT[32:48, c0:c0 + 512],
                                     start=True, stop=True)
                    nc.scalar.copy(crep[:, c0:c0 + 512], psc2[:, :])
                crw = pers.tile([128, WIN], BF16, name=f"crw_{w}", tag=f"crw_{w}")
                nc.scalar.copy(crw[:, :], crep[:, w0:w0 + WIN])

                fp_d[w] = pers.tile([128, NT], FP32, name=f"fp_{w}", tag=f"fp_{w}")
                dt0_d[w], dt1_d[w] = dt0, dt1
                dtu0_d[w], dtu1_d[w] = dtu0, dtu1
                brep_d[w], crep_d[w] = brep, crep
                srw0_d[w], srw1_d[w], crw_d[w] = srw0, srw1, crw

            if dbg:
                nc.sync.dma_start(dbg_t["dbg_dt_f"][0:128, :],
                                  dt0_d["f"][:, :].bitcast(FP32))
                nc.sync.dma_start(dbg_t["dbg_dt_f"][128:192, :],
                                  dt1_d["f"][:, :].bitcast(FP32))

            # correction kernels K = exp(A*Srel)*Crep, persistent to post-AG

            K_d = {w: [pers.tile([128, WIN], BF16, name=f"K_{w}_{k}",
                                 tag=f"K_{w}_{k}") for k in range(NT)]
                   for w in dirs}

            # AG buffers (one per direction)
            ag_in = {w: drp.tile([NT, 128], FP32, name=f"ag_in_{w}",
                                 tag=f"ag_in_{w}", space="DRAM") for w in dirs}
            ag_out = {w: drp.tile([NS, NT * 128], FP32, name=f"ag_out_{w}",
                                  tag=f"ag_out_{w}", space="DRAM",
                                  addr_space="Shared") for w in dirs}

            h_in_d = {}
            ycps_d = {}

            with tc.tile_pool(name="psy", bufs=1, space="PSUM") as psy:
                y_ps0 = psy.tile([128, SL], FP32, name="y_ps0", tag="y0",
                                 space="PSUM")
                y_ps1f = psy.tile([128, SL], FP32, name="y_ps1", tag="y1",
                                  space="PSUM")
                y_ps1 = y_ps1f[64:128]

                for di, w in enumerate(dirs):
                    rev = (w == "r")
                    w0 = 0 if not rev else SL - WIN
                    dt0, dt1 = dt0_d[w], dt1_d[w]
                    dtu0, dtu1 = dtu0_d[w], dtu1_d[w]
                    brep, crep = brep_d[w], crep_d[w]
                    arep = cf(f"Arep_{w}")

                    # ---- per-tile scan pipeline ----
                    for k in range(NT):
                        r0 = 8 * k
                        (srcdt, srcdtu) = (dt0, dtu0) if r0 < 128 else (dt1, dtu1)
                        ro = r0 if r0 < 128 else r0 - 128
                        q0 = (ro // 64) * 64
                        nq = min(64, (128 if r0 < 128 else 64) - q0)
                        oq = q0 if r0 < 128 else 64   # ohs slice base matches rhs
                        v = (ro % 64) // 8
                        ohs = cfr(f"ohs{v}")
                        dA = wk.tile([128, SL], FP32, name="dA", tag="dA", bufs=2)
                        for c0 in (0, 512):
                            rp = psw.tile([128, 512], FP32, name="rep_ps", tag="w",
                                          space="PSUM")
                            nc.tensor.matmul(rp[:, :],
                                             ohs[oq:oq + nq, :],
                                             srcdt[q0:q0 + nq, c0:c0 + 512],
                                             start=True, stop=True)
                            nc.scalar.activation(dA[:, c0:c0 + 512], rp[:, :], AF.Exp,
                                                 scale=arep[:, k:k + 1])
                        dBu = wk.tile([128, SL], FP32, name="dBu", tag="dBu", bufs=2)
                        for c0 in (0, 512):
                            rp = psw.tile([128, 512], FP32, name="rep2_ps", tag="w",
                                          space="PSUM")
                            nc.tensor.matmul(rp[:, :],
                                             ohs[oq:oq + nq, :],
                                             srcdtu[q0:q0 + nq, c0:c0 + 512],
                                             start=True, stop=True)
                            if k % 2 == 0:
                                nc.vector.tensor_mul(dBu[:, c0:c0 + 512], rp[:, :],
                                                     brep[:, c0:c0 + 512])
                            else:
                                # route via ACT + Pool to unload the DVE
                                dtur = wk.tile([128, 512], FP32, name="dtur",
                                               tag="dtur", bufs=3)
                                nc.scalar.copy(dtur[:, :], rp[:, :])
                                nc.gpsimd.tensor_mul(dBu[:, c0:c0 + 512],
                                                     dtur[:, :],
                                                     brep[:, c0:c0 + 512])
                        h = wk.tile([128, SL], FP32, name="h", tag="h", bufs=2)
                        if rev:
                            nc.vector.tensor_tensor_scan(h[:, ::-1], dA[:, ::-1],
                                                         dBu[:, ::-1], 0.0,
                                                         OP.mult, OP.add)
                        else:
                            nc.vector.tensor_tensor_scan(h[:, :], dA[:, :], dBu[:, :],
                                                         0.0, OP.mult, OP.add)
                        fcol = SL - 1 if not rev else 0
                        nc.vector.tensor_copy(fp_d[w][:, k:k + 1],
                                              h[:, fcol:fcol + 1])
                        if dbg and w == "f" and k == 0:
                            nc.sync.dma_start(dbg_t["dbg_h0_f"][:, :], h[:, :])
                        # hC = h * Crep  (Pool engine)
                        hC = wk.tile([128, SL], BF16, name="hC", tag="hC", bufs=2)
                        nc.gpsimd.tensor_mul(hC[:, :], h[:, :], crep[:, :])
                        # y reduce, full-height output (dst base must be 0)
                        yps = y_ps0 if r0 < 128 else y_ps1f
                        t = k if r0 < 128 else k - 8
                        kfirst = 0 if r0 < 128 else 16
                        klast = 15 if r0 < 128 else 23
                        redb = cb(f"red128b_{t}")
                        for c0 in (0, 512):
                            nc.tensor.matmul(yps[:, c0:c0 + 512],
                                             redb[:, :],
                                             hC[:, c0:c0 + 512],
                                             start=(di == 0 and k == kfirst),
                                             stop=(di == 1 and k == klast))

                        # correction kernel K_k = exp(A*Srel_win) * Crep_win
                        src_sr = srw0_d[w] if r0 < 128 else srw1_d[w]
                        srp = psw.tile([128, WIN], FP32, name="srp", tag="w",
                                       space="PSUM")
                        nc.tensor.matmul(srp[:, :], ohs[oq:oq + nq, :],
                                         src_sr[q0:q0 + nq, 0:WIN],
                                         start=True, stop=True)
                        cpda = wk.tile([128, WIN], BF16, name="cpda", tag="cpda",
                                       bufs=3)
                        nc.scalar.activation(cpda[:, :], srp[:, :], AF.Exp,
                                             scale=arep[:, k:k + 1])
                        nc.vector.tensor_mul(K_d[w][k][:, :], cpda[:, :],
                                             crw_d[w][:, :])

                    # ---- ship finals: transpose fp, DMA, AllGather ----
                    fpt_ps = psw.tile([NT, 128], FP32, name="fpt_ps", tag="w",
                                      space="PSUM")
                    nc.tensor.transpose(fpt_ps[:, :], fp_d[w][:, :],
                                        cf("ident")[:, :])
                    fpt_sb = wk.tile([NT, 128], FP32, name="fpt_sb", tag="fpt",
                                     bufs=2)
                    nc.scalar.copy(fpt_sb[:, :], fpt_ps[:, :])
                    nc.sync.dma_start(ag_in[w][:, :], fpt_sb[:, :])
                    nc.gpsimd.collective_compute(
                        "AllGather", mybir.AluOpType.bypass,
                        ins=[ag_in[w][:, :].opt()],
                        outs=[ag_out[w][:, :].opt()],
                        replica_groups=RG,
                    )

                    # ---- carry select: h_in[:,k] = agc_k^T @ onehot(neighbor) ----
                    agc = wk.tile([NS, NT * 128], FP32, name=f"agc_{w}",
                                  tag="agc", bufs=1)
                    for ch in range(4):
                        cw0 = ch * (NT * 32)
                        nc.sync.dma_start(agc[:, cw0:cw0 + NT * 32],
                                          ag_out[w][:, cw0:cw0 + NT * 32])
                    pscb = psc.tile([128, 512], FP32, name=f"pscb_{w}",
                                    tag=f"pscb_{w}", space="PSUM")
                    hin_ps = pscb[:, 384:384 + NT]
                    sel = cf(f"sel_{w}", (0, NS))
                    for k in range(NT):
                        nc.tensor.matmul(hin_ps[:, k:k + 1],
                                         agc[:, k * 128:(k + 1) * 128],
                                         sel[:, :], start=True, stop=True)
                    h_in = pers.tile([128, NT], FP32, name=f"h_in_{w}",
                                     tag=f"h_in_{w}")
                    nc.scalar.copy(h_in[:, :], hin_ps[:, :])
                    h_in_d[w] = h_in
                    if dbg:
                        nc.sync.dma_start(dbg_t[f"dbg_hin_{w}"][:, :], h_in[:, :])

                    # ---- windowed correction: yc = sum_k red_k(h)^T K_k ----
                    yc = pscb[:, 0:2 * WIN]
                    ycps_d[w] = yc
                    for k in range(NT):
                        r0 = 8 * k
                        t = k if r0 < 128 else k - 8
                        kfirst = 0 if r0 < 128 else 16
                        klast = 15 if r0 < 128 else 23
                        yccol = 0 if r0 < 128 else WIN
                        redh = wk.tile([128, 128], BF16, name="redh", tag="redh",
                                       bufs=3)
                        nc.vector.tensor_scalar_mul(redh[:, :],
                                                    cb(f"red128b_{t}")[:, :],
                                                    h_in[:, k:k + 1])
                        nc.tensor.matmul(yc[:, yccol:yccol + WIN],
                                         redh[:, :], K_d[w][k][:, :],
                                         start=(k == kfirst), stop=(k == klast))
                    if dbg and w == "f":
                        ycd = wk.tile([128, 2 * WIN], FP32, name="ycd", tag="ycd",
                                      bufs=1)
                        nc.scalar.copy(ycd[:, :], yc[:, :])
                        nc.sync.dma_start(dbg_t["dbg_yc_f"][:, :], ycd[:, :])

                # ---- phase A: y_sl*g, out_proj, base output (AG-independent)
                yslg0 = pers.tile([128, SL], BF16, name="yslg0", tag="yslg0")
                yslg1 = v64(pers, "yslg1", SL, "yslg1", dt=BF16)
                nc.vector.scalar_tensor_tensor(yslg0[:, :], u0f[:, :],
                                               cf("Dsum_a")[:, 0:1],
                                               y_ps0[:, :], OP.mult, OP.add)
                nc.vector.scalar_tensor_tensor(yslg1[:, :], u1f[:, :],
                                               cf("Dsum_b", (64, 128))[:, 0:1],
                                               y_ps1[:, :], OP.mult, OP.add)
            # psy closed: y PSUM banks free
            nc.vector.tensor_mul(yslg0[:, :], yslg0[:, :], g0[:, :])
            nc.vector.tensor_mul(yslg1[:, :], yslg1[:, :], g1[:, :])
            if dbg:
                yd = wk.tile([128, SL], FP32, name="yd", tag="ydmp", bufs=2)
                nc.scalar.copy(yd[:, :], yslg0[:, :])
                nc.sync.dma_start(dbg_t["dbg_yslg"][0:128, :], yd[:, :])
                yd2 = wk.tile([128, SL], FP32, name="yd2", tag="ydmp", bufs=2)
                nc.scalar.copy(yd2[64:128, :], yslg1[:, :])
                nc.sync.dma_start(dbg_t["dbg_yslg"][128:192, :], yd2[64:128, :])

            with tc.tile_pool(name="fin", bufs=1) as fnp:
                osl = fnp.tile([C, SL], BF16, name="osl", tag="osl")
                for c0 in (0, 512):
                    ps = psw.tile([C, 512], FP32, name="op_ps", tag="w",
                                  space="PSUM")
                    nc.tensor.matmul(ps[:, :], cb("outpT_a")[:, :],
                                     yslg0[:, c0:c0 + 512],
                                     start=True, stop=False)
                    nc.tensor.matmul(ps[:, :], cb("outpT_b", (64, 128))[:, :],
                                     yslg1[:, c0:c0 + 512],
                                     start=False, stop=True)
                    nc.scalar.copy(osl[:, c0:c0 + 512], ps[:, :])
                if dbg:
                    od = wk.tile([C, SL], FP32, name="od", tag="ydmp", bufs=2)
                    nc.scalar.copy(od[:, :], osl[:, :])
                    nc.sync.dma_start(dbg_t["dbg_osl"][:, :], od[:, :])
                # base (non-window) output: out = osl + x_skip
                MID = SL - 2 * WIN
                fmid = fnp.tile([C, MID], FP32, name="fmid", tag="fmid")
                nc.vector.tensor_add(fmid[:, :], osl[:, WIN:SL - WIN],
                                     x_sb[:, 3 + WIN:3 + SL - WIN])
                nc.sync.dma_start(out_t[:, WIN:SL - WIN], fmid[:, :])

                # window finalize per direction
                for w in dirs:
                    w0 = 0 if w == "f" else SL - WIN
                    yc = ycps_d[w]
                    ycg0 = fnp.tile([128, WIN], BF16, name=f"ycg0_{w}",
                                    tag=f"ycg0_{w}")
                    ycg1 = v64(fnp, f"ycg1_{w}", WIN, f"ycg1_{w}", dt=BF16)
                    nc.vector.tensor_mul(ycg0[:, :], yc[:, 0:WIN],
                                         g0[:, w0:w0 + WIN])
                    nc.vector.tensor_mul(ycg1[:, :], yc[64:128, WIN:2 * WIN],
                                         g1[:, w0:w0 + WIN])
                    dps = psw.tile([C, WIN], FP32, name="dps", tag="w",
                                   space="PSUM")
                    nc.tensor.matmul(dps[:, :], cb("outpT_a")[:, :], ycg0[:, :],
                                     start=True, stop=False)
                    nc.tensor.matmul(dps[:, :], cb("outpT_b", (64, 128))[:, :],
                                     ycg1[:, :], start=False, stop=True)
                    dsb = fnp.tile([C, WIN], BF16, name=f"dsb_{w}", tag=f"dsb_{w}")
                    nc.scalar.copy(dsb[:, :], dps[:, :])
                    # s = osq + osl = 2*osl + delta
                    swin = fnp.tile([C, WIN], BF16, name=f"swin_{w}",
                                    tag=f"swin_{w}")
                    nc.vector.scalar_tensor_tensor(swin[:, :], osl[:, w0:w0 + WIN],
                                                   2.0, dsb[:, :],
                                                   OP.mult, OP.add)
                    fps = psw.tile([C, WIN], FP32, name="fps", tag="w",
                                   space="PSUM")
                    nc.tensor.matmul(fps[:, :], cb("fuswT", (0, C))[:, :],
                                     swin[:, :], start=True, stop=True)
                    wgt = fnp.tile([C, WIN], BF16, name=f"wgt_{w}", tag=f"wgt_{w}")
                    nc.scalar.activation(wgt[:, :], fps[:, :], AF.Sigmoid,
                                         bias=cf("fusb", (0, C))[:, 0:1])
                    # out = osl + wgt*delta + skip
                    wd = fnp.tile([C, WIN], BF16, name=f"wd_{w}", tag=f"wd_{w}")
                    nc.vector.tensor_mul(wd[:, :], wgt[:, :], dsb[:, :])
                    o1 = fnp.tile([C, WIN], FP32, name=f"o1_{w}", tag=f"o1_{w}")
                    nc.vector.tensor_add(o1[:, :], wd[:, :], osl[:, w0:w0 + WIN])
                    fwin = fnp.tile([C, WIN], FP32, name=f"fwin_{w}",
                                    tag=f"fwin_{w}")
                    nc.vector.tensor_add(fwin[:, :], o1[:, :],
                                         x_sb[:, 3 + w0:3 + w0 + WIN])
                    nc.sync.dma_start(out_t[:, w0:w0 + WIN], fwin[:, :])

    nc.compile()
    return nc, dbg_t


def _host_prep(inputs):
    """Build per-core input maps (weight folds, const blobs, slices)."""
    import ml_dtypes

    f32 = np.float32
    ln_g = np.asarray(inputs["ln_g"], np.float64)
    ln_b = np.asarray(inputs["ln_b"], np.float64)
    W1 = np.asarray(inputs["in_proj_w"], np.float64)
    W1p = (W1 * ln_g[None, :])
    bW = W1 @ ln_b
    conv_w = np.asarray(inputs["conv_w"], np.float64)
    bias_u = np.asarray(inputs["conv_bias"], np.float64) + bW[:DIN] * conv_w.sum(axis=1)
    bias_z = bW[DIN:]

    x = np.asarray(inputs["x"], np.float32).reshape(C, NS * SL)

    # W1big col layout: [u0..127 | z0..63, u128..191 | pad64, z64..127
    #                    | pad64, z128..191]
    W1big = np.zeros((512, C), np.float64)
    W1big[0:128] = W1p[0:128]
    W1big[128:192] = W1p[DIN:DIN + 64]
    W1big[192:256] = W1p[128:192]
    W1big[320:384] = W1p[DIN + 64:DIN + 128]
    W1big[448:512] = W1p[DIN + 128:DIN + 192]

    def split_ab(vec192):
        """[192(,k)] -> a [128,k] rows 0:128; b [128,k] rows 64:128=128:192."""
        v = np.asarray(vec192, f32)
        if v.ndim == 1:
            v = v[:, None]
        a = np.zeros((128, v.shape[1]), f32)
        b = np.zeros((128, v.shape[1]), f32)
        a[:, :] = v[0:128]
        b[64:128, :] = v[128:192]
        return a, b

    blob = {}
    blobr = {}
    blob["convw_a"], blob["convw_b"] = split_ab(conv_w)
    blob["bias_u_a"], blob["bias_u_b"] = split_ab(bias_u)
    blob["bias_z_a"], blob["bias_z_b"] = split_ab(bias_z)
    Dsum = (np.asarray(inputs["D_f"], np.float64)
            + np.asarray(inputs["D_r"], np.float64))
    blob["Dsum_a"], blob["Dsum_b"] = split_ab(Dsum)
    fusb = np.zeros((128, 1), f32)
    fusb[0:C, 0] = np.asarray(inputs["fus_b"], f32)
    blob["fusb"] = fusb
    blob["ident"] = np.eye(128, dtype=f32)
    oh16s = np.zeros((128, 128), f32)
    for q in range(112):
        for p in range(128):
            if (q % 32) < 16 and p % 16 == q % 32:
                oh16s[q, p] = 1.0
    blobr["oh16s"] = oh16s
    for v in range(8):
        blobr[f"ohs{v}"] = np.asarray(
            [[1.0 if (q % 64) == 8 * v + p // 16 else 0.0
              for p in range(128)] for q in range(128)], f32)
    red = {}
    for t in range(16):
        red[t] = np.asarray(
            [[1.0 if j == 8 * t + p // 16 else 0.0
              for j in range(128)] for p in range(128)], f32)

    for w in ("f", "r"):
        xp = np.asarray(inputs[f"xproj_{w}"], np.float64)   # [38, 192]
        xp70 = np.zeros((70, DIN), np.float64)
        xp70[0:16] = xp[R:R + N]           # B
        xp70[32:48] = xp[R + N:R + 2 * N]  # C
        xp70[64:70] = xp[0:R]              # dt projection
        xpT = np.ascontiguousarray(xp70.T).astype(f32)      # [192, 70]
        a = np.zeros((128, 70), f32)
        b = np.zeros((128, 70), f32)
        a[:, :] = xpT[0:128]
        b[64:128, :] = xpT[128:192]
        blobr[f"xprojT_{w}_a"], blobr[f"xprojT_{w}_b"] = a, b
        dtw70 = np.zeros((128, 256), np.float64)
        dtwt = np.asarray(inputs[f"dt_w_{w}"], np.float64).T   # [6, 192]
        dtw70[64:70, 0:128] = dtwt[:, 0:128]
        dtw70[64:70, 192:256] = dtwt[:, 128:192]
        blobr[f"dtwT_{w}"] = dtw70.astype(f32)
        blob[f"dtb_{w}_a"], blob[f"dtb_{w}_b"] = split_ab(
            np.asarray(inputs[f"dt_b_{w}"], f32))
        A = -np.exp(np.asarray(inputs[f"A_log_{w}"], np.float64))  # [DIN, N]
        arep = np.zeros((128, NT), f32)
        for p in range(128):
            for k in range(NT):
                arep[p, k] = A[8 * k + p // 16, p % 16]
        blob[f"Arep_{w}"] = arep

    bblob = {}
    outpT = np.ascontiguousarray(np.asarray(inputs["out_proj_w"]).T).astype(f32)
    a = np.zeros((128, C), f32)
    b = np.zeros((128, C), f32)
    a[:, :] = outpT[0:128]
    b[64:128, :] = outpT[128:192]
    bblob["outpT_a"], bblob["outpT_b"] = a, b
    fw = np.zeros((128, C), f32)
    fw[0:C, :] = np.ascontiguousarray(np.asarray(inputs["fus_w"]).T).astype(f32)
    bblob["fuswT"] = fw
    for t in range(16):
        bblob[f"red128b_{t}"] = red[t]

    shared = {
        "mean96": np.full((C, C), 1.0 / C, f32),
        "w1T": np.ascontiguousarray(W1big.T).astype(f32),
    }

    in_maps = []
    for s in range(NS):
        m = dict(shared)
        xs = np.zeros((C, TP), f32)
        lo = s * SL - 3
        if lo < 0:
            xs[:, 3:] = x[:, 0:SL]
        else:
            xs[:, :] = x[:, lo:(s + 1) * SL]
        m["x_sl"] = xs
        pf = np.zeros((DIN, 3), f32)
        if s == 0:
            pf[:, :] = np.float32(-bW[:DIN, None])
        bl = dict(blob)
        bl["padfix_a"], bl["padfix_b"] = split_ab(pf)
        for w in ("f", "r"):
            j = s - 1 if w == "f" else s + 1
            sel = np.zeros((128, 1), f32)
            if 0 <= j < NS:
                sel[j, 0] = 1.0
            bl[f"sel_{w}"] = sel
        bf = np.zeros((128, F32_COLS), f32)
        for nm, (o, ncol) in _F32_OFF.items():
            bf[:, o:o + ncol] = bl[nm]
        m["blobf"] = bf
        br = np.zeros((128, F32R_COLS), f32)
        for nm, (o, ncol) in _F32R_OFF.items():
            br[:, o:o + ncol] = blobr[nm]
        m["blobr"] = br
        bb = np.zeros((128, BF_COLS), f32)
        for nm, (o, ncol) in _BF_OFF.items():
            bb[:, o:o + ncol] = bblob[nm]
        m["blobb"] = bb.astype(ml_dtypes.bfloat16)
        in_maps.append(m)
    return in_maps


def run_cores(inputs, dbg=False, trace=False):
    from concourse.bass_utils import run_bass_kernel_spmd
    key = ("g", dbg)
    if key not in _cache:
        _cache[key] = _build_graph(dbg=dbg)
    nc, dbg_t = _cache[key]
    in_maps = _host_prep(inputs)
    res = run_bass_kernel_spmd(nc, in_maps, core_ids=list(range(NS)), trace=trace)
    return res, dbg_t


def kernel(**inputs):
    res, _ = run_cores(inputs, dbg=False, trace=False)
    out = np.zeros((C, NS * SL), np.float32)
    for s in range(NS):
        out[:, s * SL:(s + 1) * SL] = res.results[s]["out"]
    return out.reshape(1, C, 8, 32, 32)
